# revision 29
# baseline (speedup 1.0000x reference)
"""AdjStackAttentionWeights kernel for 8 Trainium2 NeuronCores.

Computation: masked BatchNorm (training-mode stats over masked rows of the
whole tensor), normalize, 2-layer MLP (32 -> 64 relu -> 16), mask the output.

Strategy (mask compaction + fully SBUF-resident single pass):
  - Shard batch dim b across the 8 cores (data parallel).
  - ~50% of rows are masked out and produce zero output. The host gathers
    only the masked-in rows per core, pads to a pair-of-supertiles multiple
    (4096 rows), and uploads fp16 in the exact [128, 1024] SBUF tile layout
    (partition p = q*32 + s holds feature s of row-quarter q; free dim is
    two 512-row supertiles). Roughly halves both DMA and PE work.
  - The whole compacted input (~8.5 MiB/core) stays RESIDENT in SBUF: one
    HBM read total.
  - pass 1 (overlapped with the input DMA): per-partition sum via DVE
    tensor_scalar+accum and sum-of-squares via DVE tensor_tensor_reduce
    (fp16 operands -> DVE high-rate modes), folded to [32,2] partials on
    the PE -> AllReduce across the 8 cores.
  - fold: BN scale folded into W1 (W1' = diag(s)@W1); shift becomes a
    per-partition bias b1' = (beta - mean*s)@W1 + b1 applied during relu.
  - pass 2 (from SBUF): per supertile u, mm1 computes all 4 quarters' h into
    one [128, 1024] PSUM tile (two 64-contraction blockdiag(W1',W1')
    matmuls); one relu+bias copy -> fp16 h (ACT for even u, DVE for odd);
    mm2 uses a [128, 32] hidden-blockdiag W2 so ONE matmul yields two
    quarters' 16 heads; both supertiles' heads pack into a full-width
    [128, 512] psC, copied once to fp16 omega and DMA'd out (GpSimd DGE).
    mm2 is emitted one pair behind mm1 so relu latency hides under PE work.
  - b2 and the output mask/scatter are applied on the host (b2 is a
    constant [16] broadcast, same class of host work as the mask multiply).
"""

import numpy as np

B, NN, S, H, HEADS = 8, 512, 32, 64, 16
R_FULL = NN * NN   # 262144 rows per core before compaction
FD = 512           # free-dim elements per supertile quarter
QS = 4             # quarters stacked on the partition axis
ST = QS * FD       # 2048 rows per supertile
PAIR = 2 * ST      # 4096 rows per resident [128, 1024] tile
NCORES = 8
BN_EPS = 1e-5

_NC_CACHE = {}


def build_nc(ncores=NCORES, npairs=33):
    """Build (and bacc-compile) the SPMD bass program for one core."""
    import concourse.bass as bass
    import concourse.tile as tile
    from concourse import bacc, mybir

    f32 = mybir.dt.float32
    f16 = mybir.dt.float16

    nc = bacc.Bacc("TRN2", target_bir_lowering=False, debug=False,
                   num_devices=ncores)

    xt = nc.dram_tensor("xt", [npairs, 128, 2 * FD], f16, kind="ExternalInput")
    # w1f: two stacked copies of blockdiag(W1, W1) [128, 128]
    w1f = nc.dram_tensor("w1f", [128, 2 * H], f16, kind="ExternalInput")
    # w2f: hidden-blockdiag [[W2, 0], [0, W2]] [128, 2*HEADS]
    w2f = nc.dram_tensor("w2f", [128, 2 * HEADS], f16, kind="ExternalInput")
    w1r = nc.dram_tensor("w1r", [S, H], f32, kind="ExternalInput")  # raw W1
    # constant selector matrices for PE-side partition reshuffles
    qmat = nc.dram_tensor("qmat", [128, S], f32, kind="ExternalInput")
    bm32 = nc.dram_tensor("bm32", [S, 128], f32, kind="ExternalInput")
    bm64 = nc.dram_tensor("bm64", [H, 128], f32, kind="ExternalInput")
    svec = nc.dram_tensor("svec", [S, 4], f32, kind="ExternalInput")
    b1c = nc.dram_tensor("b1c", [H, 1], f32, kind="ExternalInput")
    out = nc.dram_tensor("out", [npairs, 128, FD], f16,
                         kind="ExternalOutput")

    xview = xt.ap()
    oview = out.ap()

    with tile.TileContext(nc) as tc:
        with (
            tc.tile_pool(name="wpool", bufs=1) as wpool,
            tc.tile_pool(name="glue", bufs=1) as glue,
            tc.tile_pool(name="bn", bufs=1) as bnpool,
            tc.tile_pool(name="res", bufs=1) as respool,
            tc.tile_pool(name="h", bufs=4) as hpool,
            tc.tile_pool(name="o", bufs=3) as opool,
            tc.tile_pool(name="psAB", bufs=2, space="PSUM") as psab_pool,
            tc.tile_pool(name="psC", bufs=2, space="PSUM") as psc_pool,
            tc.tile_pool(name="psG", bufs=1, space="PSUM") as psg_pool,
            tc.tile_pool(name="dram", bufs=1, space="DRAM") as dpool,
        ):
            # ---- pass 1: DMA-in everything, masked stats ------------------
            # (input DMAs are issued FIRST so the big stream starts at t~0;
            # the small weight/constant DMAs queue behind the first few on
            # SP and still land long before the fold needs them)
            # DVE bn_stats ~1.4us/pair vs ACT accum ~2.6us/pair (two passes
            # plus two 280ns accumulator reads)
            dve_pairs = [p for p in range(npairs) if p % 3 < 2]
            Td = 2 * len(dve_pairs)
            Ta = npairs - len(dve_pairs)
            bnbuf = bnpool.tile([128, 6 * Td], f32)
            # interleaved [sum, sq] per ACT pair -> one strided reduce later
            acc = bnpool.tile([128, max(2 * Ta, 2)], f32)
            sqscr = bnpool.tile([128, 2 * FD], f16)   # discarded squares
            sumscr = bnpool.tile([128, 2 * FD], f16)  # discarded copies
            xtiles = []

            def _weight_dmas():
                w1sb = wpool.tile([128, 2 * H], f16)  # 2x blockdiag(W1, W1)
                nc.sync.dma_start(w1sb[:], w1f[:])
                w2sb = wpool.tile([128, 2 * HEADS], f16)  # hidden-blockdiag
                nc.sync.dma_start(w2sb[:], w2f[:])
                w1rsb = glue.tile([S, H], f32)
                nc.sync.dma_start(w1rsb[:], w1r[:])
                qmsb = glue.tile([128, S], f32)
                nc.sync.dma_start(qmsb[:], qmat[:])
                b32sb = glue.tile([S, 128], f32)
                nc.sync.dma_start(b32sb[:], bm32[:])
                b64sb = glue.tile([H, 128], f32)
                nc.sync.dma_start(b64sb[:], bm64[:])
                svsb = glue.tile([S, 4], f32)
                nc.sync.dma_start(svsb[:], svec[:])
                b1sb = glue.tile([H, 1], f32)
                nc.sync.dma_start(b1sb[:], b1c[:])
                return w1sb, w2sb, w1rsb, qmsb, b32sb, b64sb, svsb, b1sb

            di = ai = 0
            for p in range(npairs):
                xres = respool.tile([128, 2 * FD], f16, tag=f"res{p}")
                xtiles.append(xres)
                # alternate issue queues: the SP sequencer pays ~640ns per
                # dma_start and becomes the bottleneck if it issues all 33
                eng = nc.sync if p % 2 == 0 else nc.gpsimd
                eng.dma_start(xres[:], xview[p])
                if p == 2:
                    (w1sb, w2sb, w1rsb, qmsb, b32sb, b64sb, svsb,
                     b1sb) = _weight_dmas()
                if p % 3 < 2:
                    for u in range(2):
                        t = 2 * di + u
                        nc.vector.bn_stats(bnbuf[:, 6 * t:6 * t + 6],
                                           xres[:, FD * u:FD * u + FD])
                    di += 1
                else:
                    nc.scalar.activation(
                        sqscr[:], xres[:],
                        mybir.ActivationFunctionType.Square,
                        accum_out=acc[:, 2 * ai + 1:2 * ai + 2])
                    nc.scalar.activation(
                        sumscr[:], xres[:],
                        mybir.ActivationFunctionType.Identity,
                        accum_out=acc[:, 2 * ai:2 * ai + 1])
                    ai += 1

            # preload the Sqrt activation table now (after the Square /
            # Identity stats ops) so the post-AllReduce fold hits it warm
            sqwarm = glue.tile([1, 1], f32)
            nc.scalar.activation(sqwarm[:], sqscr[0:1, 0:1],
                                 mybir.ActivationFunctionType.Sqrt)

            # convert bn_stats (count, mean, count*var) x {even, odd} and
            # the ACT accumulators into per-partition sum / sumsq
            bnv = bnbuf[:].rearrange("p (t k) -> p t k", k=6)
            means = bnv[:, :, 1:5:3]   # [128, Td, 2] (cols 1 and 4)
            cvars = bnv[:, :, 2:6:3]   # [128, Td, 2] (cols 2 and 5)
            half = float(FD // 2)

            msq = glue.tile([128, 2 * Td], f32)
            nc.vector.tensor_mul(msq[:], means, means)
            sum_means = glue.tile([128, 1], f32)
            nc.vector.tensor_reduce(sum_means[:], means,
                                    axis=mybir.AxisListType.XY,
                                    op=mybir.AluOpType.add)
            sum_msq = glue.tile([128, 1], f32)
            nc.vector.tensor_reduce(sum_msq[:], msq[:],
                                    axis=mybir.AxisListType.X,
                                    op=mybir.AluOpType.add)
            sum_cv = glue.tile([128, 1], f32)
            nc.vector.tensor_reduce(sum_cv[:], cvars,
                                    axis=mybir.AxisListType.XY,
                                    op=mybir.AluOpType.add)
            # ACT lanes: one strided reduce folds [sum, sq] x Ta -> [128, 2]
            pa = glue.tile([128, 2], f32)
            accv = acc[:, 0:2 * Ta].rearrange("p (t k) -> p k t", k=2)
            nc.vector.tensor_reduce(pa[:], accv, axis=mybir.AxisListType.X,
                                    op=mybir.AluOpType.add)
            partials = glue.tile([128, 2], f32)
            # sum(x) = half * sum(means) + act-lane sums
            nc.vector.tensor_scalar(partials[:, 0:1], sum_means[:], half,
                                    pa[:, 0:1], op0=mybir.AluOpType.mult,
                                    op1=mybir.AluOpType.add)
            # sum(x^2) = half * sum(means^2) + sum(count*var) + act-lane sq
            nc.vector.tensor_scalar(partials[:, 1:2], sum_msq[:], half,
                                    sum_cv[:], op0=mybir.AluOpType.mult,
                                    op1=mybir.AluOpType.add)
            nc.vector.tensor_add(partials[:, 1:2], partials[:, 1:2],
                                 pa[:, 1:2])

            # fold the 4 partition quarters on the PE: local = Q.T @ partials
            ps_st = psg_pool.tile([S, 2], f32, tag="psg")
            nc.tensor.matmul(ps_st[:], qmsb[:], partials[:], start=True,
                             stop=True, tile_position=(0, 0))
            local = glue.tile([S, 2], f32)
            nc.vector.tensor_copy(local[:], ps_st[:])

            # ---- AllReduce of [32,2] masked sums across cores -------------
            ar_in = dpool.tile([S, 2], f32)
            ar_out = dpool.tile([S, 2], f32)
            nc.gpsimd.dma_start(ar_in[:], local[:])
            nc.gpsimd.collective_compute(
                "AllReduce",
                mybir.AluOpType.add,
                replica_groups=[list(range(ncores))],
                ins=[ar_in.opt()],
                outs=[ar_out.opt()],
            )
            gst = glue.tile([S, 2], f32)
            nc.gpsimd.dma_start(gst[:], ar_out[:])

            # ---- fold stats into weights ----------------------------------
            # [sum, sumsq] * inv_cnt -> [mean, E[x^2]] in one op
            me = glue.tile([S, 2], f32)
            nc.vector.tensor_scalar(me[:], gst[:], svsb[:, 2:3], None,
                                    op0=mybir.AluOpType.mult)
            var = glue.tile([S, 1], f32)
            nc.vector.tensor_mul(var[:], me[:, 0:1], me[:, 0:1])
            nc.vector.tensor_sub(var[:], me[:, 1:2], var[:])
            nc.vector.tensor_scalar_add(var[:], var[:], BN_EPS)
            recip = glue.tile([S, 1], f32)
            nc.vector.reciprocal(recip[:], var[:])
            rstd = glue.tile([S, 1], f32)
            nc.scalar.activation(rstd[:], recip[:],
                                 mybir.ActivationFunctionType.Sqrt)
            sg = glue.tile([S, 1], f32)
            nc.vector.tensor_mul(sg[:], rstd[:], svsb[:, 0:1])    # s=gamma*rstd
            tv = glue.tile([S, 1], f32)
            nc.vector.tensor_mul(tv[:], me[:, 0:1], sg[:])
            nc.vector.tensor_sub(tv[:], svsb[:, 1:2], tv[:])      # t=beta-mean*s
            # b1' = W1.T @ t + b1
            b1p = psg_pool.tile([H, 1], f32, tag="psg")
            nc.tensor.matmul(b1p[:], w1rsb[:], tv[:], start=True,
                             stop=True, tile_position=(0, 0))
            b1f = glue.tile([H, 1], f32)
            nc.vector.tensor_add(b1f[:], b1p[:], b1sb[:])

            # broadcast b1' and s to [128,1] via PE selector matmuls
            ps_b = psg_pool.tile([128, 1], f32, tag="psg")
            nc.tensor.matmul(ps_b[:], b64sb[:], b1f[:], start=True,
                             stop=True, tile_position=(0, 0))
            bias128 = wpool.tile([128, 1], f32)
            nc.vector.tensor_copy(bias128[:], ps_b[:])
            ps_s = psg_pool.tile([128, 1], f32, tag="psg")
            nc.tensor.matmul(ps_s[:], b32sb[:], sg[:], start=True,
                             stop=True, tile_position=(0, 0))
            s4 = wpool.tile([128, 1], f32)
            nc.vector.tensor_copy(s4[:], ps_s[:])
            # scale all four W1 copies in place: W1' = diag(s) @ W1
            nc.vector.tensor_scalar(w1sb[:], w1sb[:], s4[:], None,
                                    op0=mybir.AluOpType.mult)

            # ---- pass 2: the MLP (from resident SBUF) ---------------------
            relu = mybir.ActivationFunctionType.Relu

            def _mm1_relu(p):
                xres = xtiles[p]
                hs = []
                for u in range(2):
                    xs = xres[:, FD * u:FD * u + FD]
                    psAB = psab_pool.tile([128, 2 * FD], f32, tag="psAB")
                    # paired mm1: blockdiag(W1',W1') handles two quarters
                    # per matmul; q0q1 h -> cols 0:512, q2q3 -> 512:1024
                    nc.tensor.matmul(psAB[:, 0:FD], w1sb[0:2 * S, :],
                                     xs[0:2 * S, :], start=True, stop=True,
                                     tile_position=(0, 0))
                    nc.tensor.matmul(psAB[:, FD:2 * FD], w1sb[2 * S:128, :],
                                     xs[2 * S:128, :], start=True, stop=True,
                                     tile_position=(64, 0))
                    hU = hpool.tile([128, 2 * FD], f16, tag="hU")
                    # relu(z + b1'): one [128,1024] copy; alternate engines
                    if u == 0:
                        nc.scalar.activation(hU[:], psAB[:], relu,
                                             bias=bias128[:])
                    else:
                        nc.vector.tensor_scalar(hU[:], psAB[:], bias128[:],
                                                0.0, op0=mybir.AluOpType.add,
                                                op1=mybir.AluOpType.max)
                    hs.append(hU)
                return hs

            def _mm2_and_out(p, hs):
                # psC packs both supertiles by PARTITION: u0 -> 0:64,
                # u1 -> 64:128, so the omega copy and DMA run full-width
                psC = psc_pool.tile([128, FD], f32, tag="psC")
                for u in range(2):
                    hU = hs[u]
                    # mm2: hidden-blockdiag W2 -> two quarters' heads per
                    # matmul; supertile u fills psC[64u : 64u+64]
                    nc.tensor.matmul(psC[64 * u:64 * u + 32, :],
                                     w2sb[:, 0:32], hU[:, 0:FD],
                                     start=True, stop=True,
                                     tile_position=(0, 64 * u))
                    nc.tensor.matmul(psC[64 * u + 32:64 * u + 64, :],
                                     w2sb[:, 0:32], hU[:, FD:2 * FD],
                                     start=True, stop=True,
                                     tile_position=(0, 64 * u + 32))
                omega = opool.tile([128, FD], f16, tag="om")
                if p % 2 == 0:
                    nc.vector.tensor_copy(omega[:], psC[:])
                else:
                    nc.scalar.copy(omega[:], psC[:])
                nc.gpsimd.dma_start(oview[p], omega[:])

            # mm2 emitted one pair behind mm1 so the relu latency of pair p
            # hides under pair p+1's mm1 work on the PE
            prev = None
            for p in range(npairs):
                hs = _mm1_relu(p)
                if prev is not None:
                    _mm2_and_out(p - 1, prev)
                prev = hs
            _mm2_and_out(npairs - 1, prev)

    nc.compile()
    return nc


def _get_nc(ncores, npairs):
    key = (ncores, npairs)
    if key not in _NC_CACHE:
        _NC_CACHE[key] = build_nc(ncores, npairs)
    return _NC_CACHE[key]


def make_plan(stacks, mask, gamma, beta, W1, b1, W2, b2, ncores=NCORES):
    """Host-side compaction plan: per-core masked-row indices + capacity."""
    mask = np.asarray(mask)
    idxs = [np.flatnonzero(np.asarray(mask[c]).reshape(-1))
            for c in range(ncores)]
    nmax = max((len(ix) for ix in idxs), default=0)
    npairs = max((nmax + PAIR - 1) // PAIR, 1)
    cnt = max(float(np.asarray(mask, np.float64).sum()), 1.0)
    return {"idxs": idxs, "npairs": npairs, "cnt": cnt}


def make_in_maps(plan, stacks, mask, gamma, beta, W1, b1, W2, b2,
                 ncores=NCORES):
    """Per-core input dicts (host does gather + layout transforms only)."""
    npairs = plan["npairs"]
    rows_c = npairs * PAIR
    inv_cnt = np.float32(1.0 / np.float32(plan["cnt"]))

    svec = np.zeros((S, 4), np.float32)
    svec[:, 0] = np.asarray(gamma, np.float32)
    svec[:, 1] = np.asarray(beta, np.float32)
    svec[:, 2] = inv_cnt

    qm = np.zeros((128, S), np.float32)
    qm[np.arange(128), np.arange(128) % S] = 1.0
    b32 = np.ascontiguousarray(qm.T)              # [32, 128]
    b64 = np.zeros((H, 128), np.float32)
    b64[np.arange(128) % H, np.arange(128)] = 1.0

    w1np = np.asarray(W1, np.float32)
    bd = np.zeros((2 * S, 2 * H), np.float32)     # blockdiag(W1, W1)
    bd[:S, :H] = w1np
    bd[S:, H:] = w1np
    w1f = np.tile(bd, (2, 1)).astype(np.float16)  # [128, 128]
    w2np = np.asarray(W2, np.float32)
    w2f = np.zeros((128, 2 * HEADS), np.float16)  # [[W2,0],[0,W2]] on hidden
    w2f[:H, :HEADS] = w2np.astype(np.float16)
    w2f[H:, HEADS:] = w2np.astype(np.float16)
    b1cc = np.asarray(b1, np.float32).reshape(H, 1)

    in_maps = []
    for c in range(ncores):
        idx = plan["idxs"][c]
        xbuf = np.zeros((rows_c, S), np.float16)
        xbuf[:len(idx)] = np.asarray(stacks[c], np.float32).reshape(-1, S)[idx]
        # row r = ((pair*2 + u)*4 + q)*512 + j ; partition p = q*32 + s
        v = xbuf.reshape(npairs, 2, QS, FD, S)     # [pair, u, q, j, s]
        v = v.transpose(0, 2, 4, 1, 3)             # [pair, q, s, u, j]
        xti = np.ascontiguousarray(v).reshape(npairs, 128, 2 * FD)
        in_maps.append({
            "xt": xti, "w1f": w1f, "w2f": w2f, "w1r": w1np,
            "svec": svec, "b1c": b1cc,
            "qmat": qm, "bm32": b32, "bm64": b64,
        })
    return in_maps


def assemble_output(plan, results, b2, ncores=NCORES):
    npairs = plan["npairs"]
    rows_c = npairs * PAIR
    b2f = np.asarray(b2, np.float32).reshape(1, HEADS)
    outs = []
    for c in range(ncores):
        o = results[c]["out"].astype(np.float32)   # [npairs, 128, 512] fp16
        o = o.reshape(npairs, 2, QS, HEADS, FD)    # [pair, u, q, h, j]
        o = o.transpose(0, 1, 2, 4, 3)             # [pair, u, q, j, h]
        o = np.ascontiguousarray(o).reshape(rows_c, HEADS)
        idx = plan["idxs"][c]
        full = np.zeros((R_FULL, HEADS), np.float32)
        full[idx] = o[:len(idx)] + b2f
        outs.append(full)
    return np.stack(outs)                          # [ncores, R_FULL, 16]


def kernel(stacks, mask, gamma, beta, W1, b1, W2, b2):
    from concourse.bass_utils import run_bass_kernel_spmd

    plan = make_plan(stacks, mask, gamma, beta, W1, b1, W2, b2)
    nc = _get_nc(NCORES, plan["npairs"])
    in_maps = make_in_maps(plan, stacks, mask, gamma, beta, W1, b1, W2, b2)
    res = run_bass_kernel_spmd(nc, in_maps, list(range(NCORES)))
    out = assemble_output(plan, res.results, b2)
    return out.reshape(B, NN, NN, HEADS)
